# revision 16
# baseline (speedup 1.0000x reference)
"""HaarWavelet2D (level=2) Trainium2 kernel, v3.

Contract: kernel(x, level) with x [8, 64, 256, 256] fp32, level=2.
Returns (low_freq, high_freq), each [8, 64, 256, 256] fp32.

Sharding: data-parallel over batch - core b processes x[b] (64 channels).

v3 design (validated in model3.py):
- Host pre-converts x to bf16 and pre-splits even/odd columns:
  DRAM layout [C, 128(k), 2(rp), 2(eo), 128(j)], row = 2k+rp, col = 2j+eo.
  All on-chip ops become unit-stride (2x DVE mode); no casting DMAs.
- Vertical linear ops on the PE: t1 = D@s (drained with fused Abs -> a1),
  qL = Va@s (row-pair sum folded), qh = Vh@ch0, qlo = (0.1875*V128@S2)@s2,
  qh1 = (0.1875*V128)@ch1.  All scale factors folded into weights/drains.
- abs_max ALU op computes m = max(|d[r]|,|d[r+1]|) without materializing |d|.
- Row shift for m's odd rows via one small SBUF->SBUF DMA of d (rp0).
- Horizontal resizes elementwise with per-column weight tile w0t.
- Final 2x column upsample via stt with immediate 1/3 scalar + pad columns.
- Outputs stored in split-column layout; host re-interleaves.
"""

import sys

if "/opt/trn_rl_repo" not in sys.path:
    sys.path.insert(0, "/opt/trn_rl_repo")

import numpy as np
import ml_dtypes

BF = ml_dtypes.bfloat16

B_, C_, H_, W_ = 8, 64, 256, 256
NCORES = 8
G = 4            # channels per group
NCHUNK = 2      # channels per level-0 matmul chunk (N = 2*2*128 = 512)
P = 128


# ----------------------------------------------------------------------------
# host-side weight construction
# ----------------------------------------------------------------------------

def _resize_matrix(n, N):
    M = np.zeros((N, n), dtype=np.float64)
    for i in range(N):
        c = (i + 0.5) * n / N - 0.5
        j0 = int(np.floor(c))
        f = c - j0
        M[i, min(max(j0, 0), n - 1)] += 1.0 - f
        M[i, min(max(j0 + 1, 0), n - 1)] += f
    return M


def _build_weights():
    V255 = _resize_matrix(255, 256)
    V128 = _resize_matrix(128, 256)
    Sv1 = np.zeros((255, 256))
    for r in range(255):
        Sv1[r, r] = Sv1[r, r + 1] = 1.0
    S2 = np.zeros((128, 256))
    for k in range(128):
        S2[k, 2 * k] = S2[k, 2 * k + 1] = 1.0
    D = np.zeros((255, 256))
    for r in range(255):
        D[r, r] = 1.0
        D[r, r + 1] = -1.0

    Va = 0.25 * (V255 @ Sv1)                       # [256,256]
    Vh = np.zeros((256, 256))
    Vh[:, 0:255] = 0.25 * V255
    Dm = np.zeros((256, 256))
    Dm[0:255, :] = D
    Wlo = 0.1875 * (V128 @ S2)                     # [256,256]
    Wh1 = 0.1875 * V128                            # [256,128]

    w0 = np.zeros(256)
    for i in range(1, 256):
        w0[i] = V255[i, i - 1]
    w0[255] = 1.0

    w = {}
    for p in range(2):
        for rp in range(2):
            w[f"va_{p}{rp}"] = Va[p::2, rp::2].T
            w[f"vh_{p}{rp}"] = Vh[p::2, rp::2].T
            w[f"dm_{p}{rp}"] = Dm[p::2, rp::2].T
            w[f"wlo_{p}{rp}"] = Wlo[p::2, rp::2].T
        w[f"wh1_{p}"] = Wh1[p::2, :].T
    w0eo = np.stack([w0[0::2], w0[1::2]])          # [2,128]
    w["w0t"] = np.ascontiguousarray(
        np.broadcast_to(w0eo[None, None, None], (P, G, 2, 2, 128)))
    return {k: v.astype(BF) for k, v in w.items()}


_WEIGHTS = None


def _weights():
    global _WEIGHTS
    if _WEIGHTS is None:
        _WEIGHTS = _build_weights()
    return _WEIGHTS


# ----------------------------------------------------------------------------
# bass program
# ----------------------------------------------------------------------------

_NC_CACHE = {}


def build_nc(C=C_):
    key = C
    if key in _NC_CACHE:
        return _NC_CACHE[key]

    import concourse.bass as bass
    import concourse.bacc as bacc
    import concourse.tile as tile
    import concourse.mybir as mybir

    F32 = mybir.dt.float32
    BF16 = mybir.dt.bfloat16
    Alu = mybir.AluOpType
    Act = mybir.ActivationFunctionType

    nc = bacc.Bacc("TRN2", target_bir_lowering=False)
    x_d = nc.dram_tensor("x", [C, P, 2, 2, 128], BF16, kind="ExternalInput")
    wt = _weights()
    w_d = {
        name: nc.dram_tensor(name, list(arr.shape), BF16, kind="ExternalInput")
        for name, arr in wt.items()
    }
    low_d = nc.dram_tensor("low", [C, P, 2, 2, 128], BF16, kind="ExternalOutput")
    high_d = nc.dram_tensor("high", [C, P, 2, 2, 128], BF16,
                            kind="ExternalOutput")

    n_iter = C // G

    with tile.TileContext(nc) as tc:
        with (
            tc.tile_pool(name="consts", bufs=1) as consts,
            tc.tile_pool(name="qp", bufs=1) as qp,
            tc.tile_pool(name="xin", bufs=3) as xin,
            tc.tile_pool(name="mid", bufs=2) as mid,
            tc.tile_pool(name="lv1", bufs=2) as lv1,
            tc.tile_pool(name="outp", bufs=2) as outp,
            tc.tile_pool(name="ps", bufs=1, space="PSUM") as ps,
        ):
            wtile = {}
            for name, arr in wt.items():
                t = consts.tile(list(arr.shape), BF16, tag=name)
                nc.sync.dma_start(out=t, in_=w_d[name][...])
                wtile[name] = t
            w0t = wtile["w0t"]

            # persistent slotted tiles (stable addresses for one-time pad
            # memsets; slot = it % 2)
            s2t = qp.tile([P, 2, G, 2, 2, 128], BF16, tag="s")
            d2t = qp.tile([P, 2, G, 2, 2, 128], BF16, tag="d")
            dsh2t = qp.tile([P, 2, G, 2, 128], BF16, tag="dsh")
            dTL2t = qp.tile([P, 2, G, 2, 2, 128], BF16, tag="dTL")
            dTh2t = qp.tile([P, 2, G, 2, 2, 128], BF16, tag="dTh")

            # dsh partition 127 must stay 0; DMA writes only 0:127 each group
            nc.vector.memset(dsh2t, 0.0)
            for sl in range(2):
                nc.vector.memset(s2t[:, sl, :, :, 1, 127:128], 0.0)
                nc.vector.memset(d2t[:, sl, :, :, 1, 127:128], 0.0)
                nc.vector.memset(dTL2t[:, sl, :, :, 0, 0:1], 0.0)
                nc.vector.memset(dTh2t[:, sl, :, :, 0, 0:1], 0.0)

            # PSUM: 8 banks as 8 [128,512] f32 tiles
            psA = [ps.tile([P, 512], F32, tag=f"psA{i}", name=f"psA{i}")
                   for i in range(4)]
            psB = [ps.tile([P, 512], F32, tag=f"psB{i}", name=f"psB{i}")
                   for i in range(4)]

            def stage_load(it):
                c0 = it * G
                X = xin.tile([P, G, 2, 2, 128], BF16, tag="X")
                nc.sync.dma_start(
                    out=X,
                    in_=x_d[c0:c0 + G].rearrange("c k p e j -> k c p e j"))
                return X

            def mm_quant(wpfx, rhs_of, nchunks, psl, accum_rp=True):
                """Emit LDW-efficient matmuls: for p, rp: for chunk."""
                for p in range(2):
                    if accum_rp:
                        for rp in range(2):
                            for ci in range(nchunks):
                                nc.tensor.matmul(
                                    out=psl[p * nchunks + ci][:, 0:512],
                                    lhsT=wtile[f"{wpfx}_{p}{rp}"][...],
                                    rhs=rhs_of(ci, rp),
                                    start=(rp == 0), stop=(rp == 1))
                    else:
                        for ci in range(nchunks):
                            nc.tensor.matmul(
                                out=psl[p * nchunks + ci][:, 0:512],
                                lhsT=wtile[f"{wpfx}_{p}"][...],
                                rhs=rhs_of(ci, 0),
                                start=True, stop=True)

            def stage_front(it, X):
                sl = it % 2
                s = s2t[:, sl]
                d = d2t[:, sl]
                dsh = dsh2t[:, sl]

                # s/d column-pair sums/diffs, split-column layout
                nc.vector.tensor_tensor(
                    out=s[:, :, :, 0], in0=X[:, :, :, 0], in1=X[:, :, :, 1],
                    op=Alu.add)
                nc.vector.tensor_tensor(
                    out=s[:, :, :, 1, 0:127], in0=X[:, :, :, 1, 0:127],
                    in1=X[:, :, :, 0, 1:128], op=Alu.add)
                nc.vector.tensor_tensor(
                    out=d[:, :, :, 0], in0=X[:, :, :, 0], in1=X[:, :, :, 1],
                    op=Alu.subtract)
                nc.vector.tensor_tensor(
                    out=d[:, :, :, 1, 0:127], in0=X[:, :, :, 1, 0:127],
                    in1=X[:, :, :, 0, 1:128], op=Alu.subtract)

                # ad = |d| (scalar), row-shifted rp0 slice for the odd rows
                ad = mid.tile([P, G, 2, 2, 128], BF16, tag="ad")
                nc.scalar.activation(out=ad, in_=d, func=Act.Abs)
                nc.sync.dma_start(out=dsh[0:127], in_=ad[1:128, :, 0])

                # t1 = D @ s on PE, drained with Abs*0.5 -> a1
                a1 = mid.tile([P, G, 2, 2, 128], BF16, tag="a1")
                mm_quant("dm", lambda ci, rp: s2t[:, sl, 2 * ci:2 * ci + 2, rp],
                         NCHUNK, psA)
                for p in range(2):
                    for ci in range(NCHUNK):
                        nc.scalar.activation(
                            out=a1[:, 2 * ci:2 * ci + 2, p],
                            in_=psA[p * NCHUNK + ci][:, 0:512].rearrange(
                                "r (c e j) -> r c e j", c=2, e=2),
                            func=Act.Abs, scale=0.5)

                m = mid.tile([P, G, 2, 2, 128], BF16, tag="m")
                nc.vector.tensor_tensor(
                    out=m[:, :, 0], in0=ad[:, :, 0], in1=ad[:, :, 1],
                    op=Alu.max)
                nc.vector.tensor_tensor(
                    out=m[:, :, 1], in0=ad[:, :, 1], in1=dsh, op=Alu.max)

                ch0 = mid.tile([P, G, 2, 2, 128], BF16, tag="ch0")
                nc.vector.tensor_tensor(out=ch0, in0=a1, in1=m, op=Alu.add)
                return ch0

            def stage_vert(it, ch0):
                sl = it % 2
                qL = mid.tile([P, G, 2, 2, 128], BF16, tag="qL")
                qh = mid.tile([P, G, 2, 2, 128], BF16, tag="qh")
                mm_quant("va", lambda ci, rp: s2t[:, sl, 2 * ci:2 * ci + 2, rp],
                         NCHUNK, psB)
                for p in range(2):
                    for ci in range(NCHUNK):
                        nc.scalar.copy(
                            out=qL[:, 2 * ci:2 * ci + 2, p],
                            in_=psB[p * NCHUNK + ci][:, 0:512].rearrange(
                                "r (c e j) -> r c e j", c=2, e=2))
                mm_quant("vh", lambda ci, rp: ch0[:, 2 * ci:2 * ci + 2, rp],
                         NCHUNK, psB)
                for p in range(2):
                    for ci in range(NCHUNK):
                        nc.scalar.copy(
                            out=qh[:, 2 * ci:2 * ci + 2, p],
                            in_=psB[p * NCHUNK + ci][:, 0:512].rearrange(
                                "r (c e j) -> r c e j", c=2, e=2))
                return qL, qh

            def stage_horiz(it, qL, qh):
                sl = it % 2
                L0x = mid.tile([P, G, 2, 2, 128], BF16, tag="L0x")
                h0x = mid.tile([P, G, 2, 2, 128], BF16, tag="h0x")
                for q, dT, tmptag, out in (
                    (qL, dTL2t[:, sl], "tmpL", L0x),
                    (qh, dTh2t[:, sl], "tmpH", h0x),
                ):
                    nc.vector.tensor_tensor(
                        out=dT[:, :, :, 0, 1:128], in0=q[:, :, :, 1, 0:127],
                        in1=q[:, :, :, 0, 1:128], op=Alu.subtract)
                    nc.vector.tensor_tensor(
                        out=dT[:, :, :, 1], in0=q[:, :, :, 0],
                        in1=q[:, :, :, 1], op=Alu.subtract)
                    tmp = mid.tile([P, G, 2, 2, 128], BF16, tag=tmptag)
                    nc.vector.tensor_tensor(out=tmp, in0=dT, in1=w0t,
                                            op=Alu.mult)
                    nc.vector.tensor_tensor(out=out, in0=q, in1=tmp,
                                            op=Alu.add)
                return L0x, h0x

            def stage_lv1_front(it, L0x):
                s2 = lv1.tile([P, G, 2, 128], BF16, tag="s2")
                d2 = lv1.tile([P, G, 2, 128], BF16, tag="d2")
                nc.vector.tensor_tensor(
                    out=s2, in0=L0x[:, :, :, 0], in1=L0x[:, :, :, 1],
                    op=Alu.add)
                nc.vector.tensor_tensor(
                    out=d2, in0=L0x[:, :, :, 0], in1=L0x[:, :, :, 1],
                    op=Alu.subtract)
                t1b = lv1.tile([P, G, 128], BF16, tag="t1b")
                a1b = lv1.tile([P, G, 128], BF16, tag="a1b")
                ad2 = lv1.tile([P, G, 2, 128], BF16, tag="ad2")
                m1 = lv1.tile([P, G, 128], BF16, tag="m1")
                ch1 = lv1.tile([P, G, 128], BF16, tag="ch1")
                nc.vector.tensor_tensor(
                    out=t1b, in0=s2[:, :, 0], in1=s2[:, :, 1], op=Alu.subtract)
                nc.scalar.activation(out=a1b, in_=t1b, func=Act.Abs, scale=0.5)
                nc.scalar.activation(out=ad2, in_=d2, func=Act.Abs)
                nc.vector.tensor_tensor(
                    out=m1, in0=ad2[:, :, 0], in1=ad2[:, :, 1], op=Alu.max)
                nc.vector.tensor_tensor(out=ch1, in0=a1b, in1=m1, op=Alu.add)
                return s2, ch1

            def stage_lv1_mm(it, s2, ch1):
                qlo = lv1.tile([P, G, 2, 130], BF16, tag="qlo")
                qh1 = lv1.tile([P, G, 2, 130], BF16, tag="qh1")
                mm_quant("wlo", lambda ci, rp: s2[:, :, rp], 1, psA[0:2])
                mm_quant("wh1", lambda ci, rp: ch1, 1, psA[2:4],
                         accum_rp=False)
                for p in range(2):
                    for src, dst in ((psA[p], qlo), (psA[2 + p], qh1)):
                        sv = src[:, 0:512].rearrange("r (g j) -> r g j", j=128)
                        nc.scalar.copy(out=dst[:, :, p, 1:129], in_=sv)
                        # both edge pad cols in one strided drain
                        nc.scalar.copy(out=dst[:, :, p, 0:130:129],
                                       in_=sv[:, :, 0:128:127])
                return qlo, qh1

            def stage_final(it, qlo, qh1, h0x):
                c0 = it * G
                low = outp.tile([P, G, 2, 2, 128], BF16, tag="low")
                hh = outp.tile([P, G, 2, 2, 128], BF16, tag="hh")
                high = outp.tile([P, G, 2, 2, 128], BF16, tag="high")
                for q, out in ((qlo, low), (qh1, hh)):
                    nc.vector.scalar_tensor_tensor(
                        out=out[:, :, :, 0], in0=q[:, :, :, 0:128],
                        scalar=1.0 / 3.0, in1=q[:, :, :, 1:129],
                        op0=Alu.mult, op1=Alu.add)
                    nc.vector.scalar_tensor_tensor(
                        out=out[:, :, :, 1], in0=q[:, :, :, 2:130],
                        scalar=1.0 / 3.0, in1=q[:, :, :, 1:129],
                        op0=Alu.mult, op1=Alu.add)
                nc.gpsimd.tensor_tensor(out=high, in0=h0x, in1=hh, op=Alu.add)
                nc.scalar.dma_start(
                    out=low_d[c0:c0 + G].rearrange("c k p e j -> k c p e j"),
                    in_=low)
                nc.scalar.dma_start(
                    out=high_d[c0:c0 + G].rearrange("c k p e j -> k c p e j"),
                    in_=high)

            # Skewed-stage pipeline: in emission slot `it`,
            #   front+vert of group it, horiz+lv1_front of it-1,
            #   lv1_mm+final of it-2 -- so every engine's in-order queue
            # only holds work whose inputs finished >=1 slot ago.
            xt = {k: stage_load(k) for k in range(2)}
            qLh = {}
            mid1 = {}
            for it in range(n_iter + 2):
                if it < n_iter:
                    if it + 2 < n_iter:
                        xt[it + 2] = stage_load(it + 2)
                    ch0 = stage_front(it, xt.pop(it))
                    qLh[it] = stage_vert(it, ch0)
                if 1 <= it <= n_iter:
                    qL, qh = qLh.pop(it - 1)
                    L0x, h0x = stage_horiz(it - 1, qL, qh)
                    s2, ch1 = stage_lv1_front(it - 1, L0x)
                    mid1[it - 1] = (s2, ch1, h0x)
                if it >= 2:
                    s2, ch1, h0x = mid1.pop(it - 2)
                    qlo, qh1 = stage_lv1_mm(it - 2, s2, ch1)
                    stage_final(it - 2, qlo, qh1, h0x)

    nc.compile()
    _NC_CACHE[key] = nc
    return nc


# ----------------------------------------------------------------------------
# host entry points
# ----------------------------------------------------------------------------

def _host_prep(x):
    """x [B, C, H, W] f32 -> [B*C, 128, 2, 2, 128] bf16 split-column."""
    xb = x.reshape(B_ * C_, P, 2, 128, 2).astype(BF)
    return np.ascontiguousarray(xb.transpose(0, 1, 2, 4, 3))


def _host_post(out):
    """[B*C, 128, 2, 2, 128] bf16 -> [B, C, H, W] f32 interleaved."""
    o = np.asarray(out).transpose(0, 1, 2, 4, 3)
    return np.ascontiguousarray(o).reshape(B_, C_, H_, W_).astype(np.float32)


_RUNNER = None


def _get_runner():
    """Builds (once) a cached sharded jit executable over the 8 cores."""
    global _RUNNER
    if _RUNNER is not None:
        return _RUNNER

    import jax
    from jax.sharding import Mesh, PartitionSpec, NamedSharding
    from jax.experimental.shard_map import shard_map
    import concourse.mybir as mybir
    from concourse import bass2jax
    from concourse.bass2jax import _bass_exec_p, partition_id_tensor

    bass2jax.install_neuronx_cc_hook()
    nc = build_nc(C_)

    partition_name = nc.partition_id_tensor.name if nc.partition_id_tensor else None
    in_names, out_names, out_avals = [], [], []
    for alloc in nc.m.functions[0].allocations:
        if not isinstance(alloc, mybir.MemoryLocationSet):
            continue
        name = alloc.memorylocations[0].name
        if alloc.kind == "ExternalInput":
            if name != partition_name:
                in_names.append(name)
        elif alloc.kind == "ExternalOutput":
            out_names.append(name)
            out_avals.append(jax.core.ShapedArray(
                tuple(alloc.tensor_shape), mybir.dt.np(alloc.dtype)))
    n_params = len(in_names)
    all_in_names = list(in_names) + list(out_names)
    if partition_name is not None:
        all_in_names.append(partition_name)

    def _body(*args):
        operands = list(args)
        if partition_name is not None:
            operands.append(partition_id_tensor())
        return tuple(_bass_exec_p.bind(
            *operands,
            out_avals=tuple(out_avals),
            in_names=tuple(all_in_names),
            out_names=tuple(out_names),
            lowering_input_output_aliases=(),
            sim_require_finite=True,
            sim_require_nnan=True,
            nc=nc,
        ))

    devices = jax.devices()[:NCORES]
    mesh = Mesh(np.asarray(devices), ("core",))
    n_in = n_params + len(out_names)
    sharded = jax.jit(shard_map(
        _body, mesh=mesh,
        in_specs=(PartitionSpec("core"),) * n_in,
        out_specs=(PartitionSpec("core"),) * len(out_names),
        check_rep=False))

    shard0 = NamedSharding(mesh, PartitionSpec("core"))
    wt = _weights()
    static = {}
    for name in in_names:
        if name == "x":
            continue
        arr = np.concatenate([wt[name]] * NCORES, axis=0)
        static[name] = jax.device_put(arr, shard0)
    for name, aval in zip(out_names, out_avals):
        z = np.zeros((aval.shape[0] * NCORES,) + tuple(aval.shape[1:]),
                     dtype=aval.dtype)
        static[name] = jax.device_put(z, shard0)

    def run(x_global):
        ops = []
        for name in in_names:
            ops.append(x_global if name == "x" else static[name])
        for name in out_names:
            ops.append(static[name])
        outs = sharded(*ops)
        return dict(zip(out_names, outs))

    _RUNNER = (run, shard0)
    return _RUNNER


def _run_device(x, trace=False):
    """x: [8, 64, 256, 256] fp32. Returns (low, high, results_obj)."""
    xb = _host_prep(x)
    if trace:
        from concourse import bass_utils
        nc = build_nc(C_)
        wt = _weights()
        in_maps = [
            dict(wt, x=xb[b * C_:(b + 1) * C_])
            for b in range(NCORES)
        ]
        res = bass_utils.run_bass_kernel_spmd(
            nc, in_maps, core_ids=list(range(NCORES)), trace=True)
        low = np.stack([res.results[b]["low"] for b in range(NCORES)])
        high = np.stack([res.results[b]["high"] for b in range(NCORES)])
        low = low.reshape(B_ * C_, P, 2, 2, 128)
        high = high.reshape(B_ * C_, P, 2, 2, 128)
        return _host_post(low), _host_post(high), res

    run, _ = _get_runner()
    outs = run(xb)
    return (_host_post(outs["low"]), _host_post(outs["high"]), None)


def _fallback(x, level):
    """Numpy port of the reference for unexpected shapes/levels."""
    xl = x.astype(np.float64)
    low = xl
    high = np.zeros_like(xl)
    Bb, Cc, H, W = xl.shape

    def up(a, n_r, n_c):
        Mr = _resize_matrix(a.shape[-2], n_r)
        Mc = _resize_matrix(a.shape[-1], n_c)
        return np.einsum("ij,...jk,lk->...il", Mr, a, Mc)

    for lv in range(level):
        stride = 2 ** lv
        if H // stride < 2 or W // stride < 2:
            break
        x00 = low[..., 0:H - 1:stride, 0:W - 1:stride]
        x01 = low[..., 0:H - 1:stride, 1:W:stride]
        x10 = low[..., 1:H:stride, 0:W - 1:stride]
        x11 = low[..., 1:H:stride, 1:W:stride]
        ll = (x00 + x01 + x10 + x11) * 0.25
        lh = (x00 + x01 - x10 - x11) * 0.25
        hl = (x00 - x01 + x10 - x11) * 0.25
        hh = (x00 - x01 - x10 + x11) * 0.25
        ch = np.abs(lh) + np.abs(hl) + np.abs(hh)
        high = high + up(ch, H, W)
        low = up(ll, H, W)
    if level > 0:
        high = high / level
    return low.astype(np.float32), high.astype(np.float32)


def kernel(x, level):
    x = np.asarray(x, dtype=np.float32)
    level = int(level)
    if level != 2 or x.shape != (B_, C_, H_, W_):
        return _fallback(x, level)
    low, high, _ = _run_device(x)
    return low, high


# revision 21
# speedup vs baseline: 1.0831x; 1.0831x over previous
"""HaarWavelet2D (level=2) Trainium2 kernel, v3.

Contract: kernel(x, level) with x [8, 64, 256, 256] fp32, level=2.
Returns (low_freq, high_freq), each [8, 64, 256, 256] fp32.

Sharding: data-parallel over batch - core b processes x[b] (64 channels).

v3 design (validated in model3.py):
- Host pre-converts x to bf16 and pre-splits even/odd columns:
  DRAM layout [C, 128(k), 2(rp), 2(eo), 128(j)], row = 2k+rp, col = 2j+eo.
  All on-chip ops become unit-stride (2x DVE mode); no casting DMAs.
- Vertical linear ops on the PE: t1 = D@s (drained with fused Abs -> a1),
  qL = Va@s (row-pair sum folded), qh = Vh@ch0, qlo = (0.1875*V128@S2)@s2,
  qh1 = (0.1875*V128)@ch1.  All scale factors folded into weights/drains.
- abs_max ALU op computes m = max(|d[r]|,|d[r+1]|) without materializing |d|.
- Row shift for m's odd rows via one small SBUF->SBUF DMA of d (rp0).
- Horizontal resizes elementwise with per-column weight tile w0t.
- Final 2x column upsample via stt with immediate 1/3 scalar + pad columns.
- Outputs stored in split-column layout; host re-interleaves.
"""

import sys

if "/opt/trn_rl_repo" not in sys.path:
    sys.path.insert(0, "/opt/trn_rl_repo")

import numpy as np
import ml_dtypes

BF = ml_dtypes.bfloat16

B_, C_, H_, W_ = 8, 64, 256, 256
NCORES = 8
G = 4            # channels per group
NCHUNK = 2      # channels per level-0 matmul chunk (N = 2*2*128 = 512)
P = 128


# ----------------------------------------------------------------------------
# host-side weight construction
# ----------------------------------------------------------------------------

def _resize_matrix(n, N):
    M = np.zeros((N, n), dtype=np.float64)
    for i in range(N):
        c = (i + 0.5) * n / N - 0.5
        j0 = int(np.floor(c))
        f = c - j0
        M[i, min(max(j0, 0), n - 1)] += 1.0 - f
        M[i, min(max(j0 + 1, 0), n - 1)] += f
    return M


def _build_weights():
    V255 = _resize_matrix(255, 256)
    V128 = _resize_matrix(128, 256)
    Sv1 = np.zeros((255, 256))
    for r in range(255):
        Sv1[r, r] = Sv1[r, r + 1] = 1.0
    S2 = np.zeros((128, 256))
    for k in range(128):
        S2[k, 2 * k] = S2[k, 2 * k + 1] = 1.0
    D = np.zeros((255, 256))
    for r in range(255):
        D[r, r] = 1.0
        D[r, r + 1] = -1.0

    Va = 0.25 * (V255 @ Sv1)                       # [256,256]
    Vh = np.zeros((256, 256))
    Vh[:, 0:255] = 0.25 * V255
    Dm = np.zeros((256, 256))
    Dm[0:255, :] = D
    Wlo = 0.1875 * (V128 @ S2)                     # [256,256]
    Wh1 = 0.1875 * V128                            # [256,128]

    w0 = np.zeros(256)
    for i in range(1, 256):
        w0[i] = V255[i, i - 1]
    w0[255] = 1.0

    w = {}
    for p in range(2):
        for rp in range(2):
            w[f"va_{p}{rp}"] = Va[p::2, rp::2].T
            w[f"vh_{p}{rp}"] = Vh[p::2, rp::2].T
            w[f"dm_{p}{rp}"] = Dm[p::2, rp::2].T
            w[f"wlo_{p}{rp}"] = Wlo[p::2, rp::2].T
        w[f"wh1_{p}"] = Wh1[p::2, :].T
    w0eo = np.stack([w0[0::2], w0[1::2]])          # [2,128]
    w["w0t"] = np.ascontiguousarray(
        np.broadcast_to(w0eo[None, None, None], (P, G, 2, 2, 128)))
    return {k: v.astype(BF) for k, v in w.items()}


_WEIGHTS = None


def _weights():
    global _WEIGHTS
    if _WEIGHTS is None:
        _WEIGHTS = _build_weights()
    return _WEIGHTS


# ----------------------------------------------------------------------------
# bass program
# ----------------------------------------------------------------------------

_NC_CACHE = {}


def build_nc(C=C_):
    key = C
    if key in _NC_CACHE:
        return _NC_CACHE[key]

    import concourse.bass as bass
    import concourse.bacc as bacc
    import concourse.tile as tile
    import concourse.mybir as mybir

    F32 = mybir.dt.float32
    BF16 = mybir.dt.bfloat16
    Alu = mybir.AluOpType
    Act = mybir.ActivationFunctionType

    nc = bacc.Bacc("TRN2", target_bir_lowering=False)
    x_d = nc.dram_tensor("x", [C, P, 2, 2, 128], BF16, kind="ExternalInput")
    wt = _weights()
    w_d = {
        name: nc.dram_tensor(name, list(arr.shape), BF16, kind="ExternalInput")
        for name, arr in wt.items()
    }
    low_d = nc.dram_tensor("low", [C, P, 2, 2, 128], BF16, kind="ExternalOutput")
    high_d = nc.dram_tensor("high", [C, P, 2, 2, 128], BF16,
                            kind="ExternalOutput")

    n_iter = C // G

    with tile.TileContext(nc) as tc:
        with (
            tc.tile_pool(name="consts", bufs=1) as consts,
            tc.tile_pool(name="qp", bufs=1) as qp,
            tc.tile_pool(name="xin", bufs=3) as xin,
            tc.tile_pool(name="mid", bufs=2) as mid,
            tc.tile_pool(name="lv1", bufs=2) as lv1,
            tc.tile_pool(name="outp", bufs=2) as outp,
            tc.tile_pool(name="ps", bufs=1, space="PSUM") as ps,
        ):
            wtile = {}
            for name, arr in wt.items():
                t = consts.tile(list(arr.shape), BF16, tag=name)
                nc.sync.dma_start(out=t, in_=w_d[name][...])
                wtile[name] = t
            w0t = wtile["w0t"]

            # persistent slotted tiles (stable addresses for one-time pad
            # memsets; slot = it % 2)
            s2t = qp.tile([P, 2, G, 2, 2, 128], BF16, tag="s")
            d2t = qp.tile([P, 2, G, 2, 2, 128], BF16, tag="d")
            dsh2t = qp.tile([P, 2, G, 2, 128], BF16, tag="dsh")
            dTL2t = qp.tile([P, 2, G, 2, 2, 128], BF16, tag="dTL")
            dTh2t = qp.tile([P, 2, G, 2, 2, 128], BF16, tag="dTh")

            # dsh partition 127 must stay 0; DMA writes only 0:127 each group
            nc.vector.memset(dsh2t, 0.0)
            for sl in range(2):
                nc.vector.memset(s2t[:, sl, :, :, 1, 127:128], 0.0)
                nc.vector.memset(d2t[:, sl, :, :, 1, 127:128], 0.0)
                nc.vector.memset(dTL2t[:, sl, :, :, 0, 0:1], 0.0)
                nc.vector.memset(dTh2t[:, sl, :, :, 0, 0:1], 0.0)

            # PSUM: 8 banks as 8 [128,512] f32 tiles
            psA = [ps.tile([P, 512], F32, tag=f"psA{i}", name=f"psA{i}")
                   for i in range(4)]
            psB = [ps.tile([P, 512], F32, tag=f"psB{i}", name=f"psB{i}")
                   for i in range(4)]

            def stage_load(it):
                c0 = it * G
                X = xin.tile([P, G, 2, 2, 128], BF16, tag="X")
                nc.sync.dma_start(
                    out=X,
                    in_=x_d[c0:c0 + G].rearrange("c k p e j -> k c p e j"))
                return X

            def mm_quant(wpfx, rhs_of, nchunks, psl, accum_rp=True):
                """Emit LDW-efficient matmuls: for p, rp: for chunk."""
                for p in range(2):
                    if accum_rp:
                        for rp in range(2):
                            for ci in range(nchunks):
                                nc.tensor.matmul(
                                    out=psl[p * nchunks + ci][:, 0:512],
                                    lhsT=wtile[f"{wpfx}_{p}{rp}"][...],
                                    rhs=rhs_of(ci, rp),
                                    start=(rp == 0), stop=(rp == 1))
                    else:
                        for ci in range(nchunks):
                            nc.tensor.matmul(
                                out=psl[p * nchunks + ci][:, 0:512],
                                lhsT=wtile[f"{wpfx}_{p}"][...],
                                rhs=rhs_of(ci, 0),
                                start=True, stop=True)

            def stage_front_a(it, X):
                sl = it % 2
                s = s2t[:, sl]
                d = d2t[:, sl]
                dsh = dsh2t[:, sl]

                # s/d column-pair sums/diffs, split-column layout
                nc.vector.tensor_tensor(
                    out=s[:, :, :, 0], in0=X[:, :, :, 0], in1=X[:, :, :, 1],
                    op=Alu.add)
                nc.vector.tensor_tensor(
                    out=s[:, :, :, 1, 0:127], in0=X[:, :, :, 1, 0:127],
                    in1=X[:, :, :, 0, 1:128], op=Alu.add)
                nc.vector.tensor_tensor(
                    out=d[:, :, :, 0], in0=X[:, :, :, 0], in1=X[:, :, :, 1],
                    op=Alu.subtract)
                nc.vector.tensor_tensor(
                    out=d[:, :, :, 1, 0:127], in0=X[:, :, :, 1, 0:127],
                    in1=X[:, :, :, 0, 1:128], op=Alu.subtract)

                # ad = |d| (scalar; first op of this slot's scalar segment),
                # row-shifted rp0 slice for the odd rows
                ad = mid.tile([P, G, 2, 2, 128], BF16, tag="ad")
                nc.scalar.activation(out=ad, in_=d, func=Act.Abs)
                nc.sync.dma_start(out=dsh[0:127], in_=ad[1:128, :, 0])

                # t1 = D @ s on PE, drained with Abs*0.5 -> a1
                a1 = mid.tile([P, G, 2, 2, 128], BF16, tag="a1")
                mm_quant("dm", lambda ci, rp: s2t[:, sl, 2 * ci:2 * ci + 2, rp],
                         NCHUNK, psA)
                for p in range(2):
                    for ci in range(NCHUNK):
                        nc.scalar.activation(
                            out=a1[:, 2 * ci:2 * ci + 2, p],
                            in_=psA[p * NCHUNK + ci][:, 0:512].rearrange(
                                "r (c e j) -> r c e j", c=2, e=2),
                            func=Act.Abs, scale=0.5)
                return ad, a1

            def stage_vert_a(it):
                sl = it % 2
                qL = mid.tile([P, G, 2, 2, 128], BF16, tag="qL")
                mm_quant("va", lambda ci, rp: s2t[:, sl, 2 * ci:2 * ci + 2, rp],
                         NCHUNK, psB)
                for p in range(2):
                    for ci in range(NCHUNK):
                        nc.scalar.copy(
                            out=qL[:, 2 * ci:2 * ci + 2, p],
                            in_=psB[p * NCHUNK + ci][:, 0:512].rearrange(
                                "r (c e j) -> r c e j", c=2, e=2))
                return qL

            def stage_front_b(it, ad):
                sl = it % 2
                dsh = dsh2t[:, sl]
                m = mid.tile([P, G, 2, 2, 128], BF16, tag="m")
                nc.vector.tensor_tensor(
                    out=m[:, :, 0], in0=ad[:, :, 0], in1=ad[:, :, 1],
                    op=Alu.max)
                nc.vector.tensor_tensor(
                    out=m[:, :, 1], in0=ad[:, :, 1], in1=dsh, op=Alu.max)
                return m

            def stage_vert_b(it, a1, m):
                # qh = Vh @ (a1 + m), summed via PSUM accumulation
                qh = mid.tile([P, G, 2, 2, 128], BF16, tag="qh")
                for p in range(2):
                    for rp in range(2):
                        for ci in range(NCHUNK):
                            for si, src in enumerate((a1, m)):
                                nc.tensor.matmul(
                                    out=psB[p * NCHUNK + ci][:, 0:512],
                                    lhsT=wtile[f"vh_{p}{rp}"][...],
                                    rhs=src[:, 2 * ci:2 * ci + 2, rp],
                                    start=(rp == 0 and si == 0),
                                    stop=(rp == 1 and si == 1))
                for p in range(2):
                    for ci in range(NCHUNK):
                        nc.scalar.copy(
                            out=qh[:, 2 * ci:2 * ci + 2, p],
                            in_=psB[p * NCHUNK + ci][:, 0:512].rearrange(
                                "r (c e j) -> r c e j", c=2, e=2))
                return qh

            def stage_horiz(it, qL, qh):
                sl = it % 2
                L0x = mid.tile([P, G, 2, 2, 128], BF16, tag="L0x")
                h0x = mid.tile([P, G, 2, 2, 128], BF16, tag="h0x")
                for q, dT, tmptag, out in (
                    (qL, dTL2t[:, sl], "tmpL", L0x),
                    (qh, dTh2t[:, sl], "tmpH", h0x),
                ):
                    nc.vector.tensor_tensor(
                        out=dT[:, :, :, 0, 1:128], in0=q[:, :, :, 1, 0:127],
                        in1=q[:, :, :, 0, 1:128], op=Alu.subtract)
                    nc.vector.tensor_tensor(
                        out=dT[:, :, :, 1], in0=q[:, :, :, 0],
                        in1=q[:, :, :, 1], op=Alu.subtract)
                    tmp = mid.tile([P, G, 2, 2, 128], BF16, tag=tmptag)
                    nc.vector.tensor_tensor(out=tmp, in0=dT, in1=w0t,
                                            op=Alu.mult)
                    nc.vector.tensor_tensor(out=out, in0=q, in1=tmp,
                                            op=Alu.add)
                return L0x, h0x

            def stage_lv1_front(it, L0x):
                s2 = lv1.tile([P, G, 2, 128], BF16, tag="s2")
                d2 = lv1.tile([P, G, 2, 128], BF16, tag="d2")
                nc.vector.tensor_tensor(
                    out=s2, in0=L0x[:, :, :, 0], in1=L0x[:, :, :, 1],
                    op=Alu.add)
                nc.vector.tensor_tensor(
                    out=d2, in0=L0x[:, :, :, 0], in1=L0x[:, :, :, 1],
                    op=Alu.subtract)
                t1b = lv1.tile([P, G, 128], BF16, tag="t1b")
                a1b = lv1.tile([P, G, 128], BF16, tag="a1b")
                ad2 = lv1.tile([P, G, 2, 128], BF16, tag="ad2")
                m1 = lv1.tile([P, G, 128], BF16, tag="m1")
                ch1 = lv1.tile([P, G, 128], BF16, tag="ch1")
                nc.vector.tensor_tensor(
                    out=t1b, in0=s2[:, :, 0], in1=s2[:, :, 1], op=Alu.subtract)
                nc.scalar.activation(out=a1b, in_=t1b, func=Act.Abs, scale=0.5)
                nc.scalar.activation(out=ad2, in_=d2, func=Act.Abs)
                nc.vector.tensor_tensor(
                    out=m1, in0=ad2[:, :, 0], in1=ad2[:, :, 1], op=Alu.max)
                nc.vector.tensor_tensor(out=ch1, in0=a1b, in1=m1, op=Alu.add)
                return s2, ch1

            def stage_lv1_mm(it, s2, ch1):
                qlo = lv1.tile([P, G, 2, 130], BF16, tag="qlo")
                qh1 = lv1.tile([P, G, 2, 130], BF16, tag="qh1")
                mm_quant("wlo", lambda ci, rp: s2[:, :, rp], 1, psA[0:2])
                mm_quant("wh1", lambda ci, rp: ch1, 1, psA[2:4],
                         accum_rp=False)
                for p in range(2):
                    for src, dst in ((psA[p], qlo), (psA[2 + p], qh1)):
                        sv = src[:, 0:512].rearrange("r (g j) -> r g j", j=128)
                        nc.scalar.copy(out=dst[:, :, p, 1:129], in_=sv)
                        # both edge pad cols in one strided drain
                        nc.scalar.copy(out=dst[:, :, p, 0:130:129],
                                       in_=sv[:, :, 0:128:127])
                return qlo, qh1

            def stage_final(it, qlo, qh1, h0x):
                c0 = it * G
                low = outp.tile([P, G, 2, 2, 128], BF16, tag="low")
                hh = outp.tile([P, G, 2, 2, 128], BF16, tag="hh")
                high = outp.tile([P, G, 2, 2, 128], BF16, tag="high")
                for q, out in ((qlo, low), (qh1, hh)):
                    nc.vector.scalar_tensor_tensor(
                        out=out[:, :, :, 0], in0=q[:, :, :, 0:128],
                        scalar=1.0 / 3.0, in1=q[:, :, :, 1:129],
                        op0=Alu.mult, op1=Alu.add)
                    nc.vector.scalar_tensor_tensor(
                        out=out[:, :, :, 1], in0=q[:, :, :, 2:130],
                        scalar=1.0 / 3.0, in1=q[:, :, :, 1:129],
                        op0=Alu.mult, op1=Alu.add)
                nc.gpsimd.tensor_tensor(out=high, in0=h0x, in1=hh, op=Alu.add)
                nc.scalar.dma_start(
                    out=low_d[c0:c0 + G].rearrange("c k p e j -> k c p e j"),
                    in_=low)
                nc.scalar.dma_start(
                    out=high_d[c0:c0 + G].rearrange("c k p e j -> k c p e j"),
                    in_=high)

            # Skewed-stage pipeline: in emission slot `it`, the vector queue
            # runs [s/d(it), horiz(it-1), m(it), lv1_front(it-1),
            # final-stts(it-2)] so the ad(it)->dsh(it) chain (gpsimd+DMA)
            # and all PE/scalar drains have a queue of independent work to
            # hide behind.
            xt = {k: stage_load(k) for k in range(2)}
            qLh = {}
            mid1 = {}
            for it in range(n_iter + 2):
                if it < n_iter:
                    if it + 2 < n_iter:
                        xt[it + 2] = stage_load(it + 2)
                    ad, a1 = stage_front_a(it, xt.pop(it))
                    qL = stage_vert_a(it)
                if 1 <= it <= n_iter:
                    pqL, pqh = qLh.pop(it - 1)
                    L0x, h0x = stage_horiz(it - 1, pqL, pqh)
                if it < n_iter:
                    m = stage_front_b(it, ad)
                    qh = stage_vert_b(it, a1, m)
                    qLh[it] = (qL, qh)
                if 1 <= it <= n_iter:
                    s2, ch1 = stage_lv1_front(it - 1, L0x)
                    mid1[it - 1] = (s2, ch1, h0x)
                if it >= 2:
                    s2, ch1, h0x = mid1.pop(it - 2)
                    qlo, qh1 = stage_lv1_mm(it - 2, s2, ch1)
                    stage_final(it - 2, qlo, qh1, h0x)

    nc.compile()
    _NC_CACHE[key] = nc
    return nc


# ----------------------------------------------------------------------------
# host entry points
# ----------------------------------------------------------------------------

def _host_prep(x):
    """x [B, C, H, W] f32 -> [B*C, 128, 2, 2, 128] bf16 split-column."""
    xb = x.reshape(B_ * C_, P, 2, 128, 2).astype(BF)
    return np.ascontiguousarray(xb.transpose(0, 1, 2, 4, 3))


def _host_post(out):
    """[B*C, 128, 2, 2, 128] bf16 -> [B, C, H, W] f32 interleaved."""
    o = np.asarray(out).transpose(0, 1, 2, 4, 3)
    return np.ascontiguousarray(o).reshape(B_, C_, H_, W_).astype(np.float32)


_RUNNER = None


def _get_runner():
    """Builds (once) a cached sharded jit executable over the 8 cores."""
    global _RUNNER
    if _RUNNER is not None:
        return _RUNNER

    import jax
    from jax.sharding import Mesh, PartitionSpec, NamedSharding
    from jax.experimental.shard_map import shard_map
    import concourse.mybir as mybir
    from concourse import bass2jax
    from concourse.bass2jax import _bass_exec_p, partition_id_tensor

    bass2jax.install_neuronx_cc_hook()
    nc = build_nc(C_)

    partition_name = nc.partition_id_tensor.name if nc.partition_id_tensor else None
    in_names, out_names, out_avals = [], [], []
    for alloc in nc.m.functions[0].allocations:
        if not isinstance(alloc, mybir.MemoryLocationSet):
            continue
        name = alloc.memorylocations[0].name
        if alloc.kind == "ExternalInput":
            if name != partition_name:
                in_names.append(name)
        elif alloc.kind == "ExternalOutput":
            out_names.append(name)
            out_avals.append(jax.core.ShapedArray(
                tuple(alloc.tensor_shape), mybir.dt.np(alloc.dtype)))
    n_params = len(in_names)
    all_in_names = list(in_names) + list(out_names)
    if partition_name is not None:
        all_in_names.append(partition_name)

    def _body(*args):
        operands = list(args)
        if partition_name is not None:
            operands.append(partition_id_tensor())
        return tuple(_bass_exec_p.bind(
            *operands,
            out_avals=tuple(out_avals),
            in_names=tuple(all_in_names),
            out_names=tuple(out_names),
            lowering_input_output_aliases=(),
            sim_require_finite=True,
            sim_require_nnan=True,
            nc=nc,
        ))

    devices = jax.devices()[:NCORES]
    mesh = Mesh(np.asarray(devices), ("core",))
    n_in = n_params + len(out_names)
    sharded = jax.jit(shard_map(
        _body, mesh=mesh,
        in_specs=(PartitionSpec("core"),) * n_in,
        out_specs=(PartitionSpec("core"),) * len(out_names),
        check_rep=False))

    shard0 = NamedSharding(mesh, PartitionSpec("core"))
    wt = _weights()
    static = {}
    for name in in_names:
        if name == "x":
            continue
        arr = np.concatenate([wt[name]] * NCORES, axis=0)
        static[name] = jax.device_put(arr, shard0)
    for name, aval in zip(out_names, out_avals):
        z = np.zeros((aval.shape[0] * NCORES,) + tuple(aval.shape[1:]),
                     dtype=aval.dtype)
        static[name] = jax.device_put(z, shard0)

    def run(x_global):
        ops = []
        for name in in_names:
            ops.append(x_global if name == "x" else static[name])
        for name in out_names:
            ops.append(static[name])
        outs = sharded(*ops)
        return dict(zip(out_names, outs))

    _RUNNER = (run, shard0)
    return _RUNNER


def _run_device(x, trace=False):
    """x: [8, 64, 256, 256] fp32. Returns (low, high, results_obj)."""
    xb = _host_prep(x)
    if trace:
        from concourse import bass_utils
        nc = build_nc(C_)
        wt = _weights()
        in_maps = [
            dict(wt, x=xb[b * C_:(b + 1) * C_])
            for b in range(NCORES)
        ]
        res = bass_utils.run_bass_kernel_spmd(
            nc, in_maps, core_ids=list(range(NCORES)), trace=True)
        low = np.stack([res.results[b]["low"] for b in range(NCORES)])
        high = np.stack([res.results[b]["high"] for b in range(NCORES)])
        low = low.reshape(B_ * C_, P, 2, 2, 128)
        high = high.reshape(B_ * C_, P, 2, 2, 128)
        return _host_post(low), _host_post(high), res

    run, _ = _get_runner()
    outs = run(xb)
    return (_host_post(outs["low"]), _host_post(outs["high"]), None)


def _fallback(x, level):
    """Numpy port of the reference for unexpected shapes/levels."""
    xl = x.astype(np.float64)
    low = xl
    high = np.zeros_like(xl)
    Bb, Cc, H, W = xl.shape

    def up(a, n_r, n_c):
        Mr = _resize_matrix(a.shape[-2], n_r)
        Mc = _resize_matrix(a.shape[-1], n_c)
        return np.einsum("ij,...jk,lk->...il", Mr, a, Mc)

    for lv in range(level):
        stride = 2 ** lv
        if H // stride < 2 or W // stride < 2:
            break
        x00 = low[..., 0:H - 1:stride, 0:W - 1:stride]
        x01 = low[..., 0:H - 1:stride, 1:W:stride]
        x10 = low[..., 1:H:stride, 0:W - 1:stride]
        x11 = low[..., 1:H:stride, 1:W:stride]
        ll = (x00 + x01 + x10 + x11) * 0.25
        lh = (x00 + x01 - x10 - x11) * 0.25
        hl = (x00 - x01 + x10 - x11) * 0.25
        hh = (x00 - x01 - x10 + x11) * 0.25
        ch = np.abs(lh) + np.abs(hl) + np.abs(hh)
        high = high + up(ch, H, W)
        low = up(ll, H, W)
    if level > 0:
        high = high / level
    return low.astype(np.float32), high.astype(np.float32)


def kernel(x, level):
    x = np.asarray(x, dtype=np.float32)
    level = int(level)
    if level != 2 or x.shape != (B_, C_, H_, W_):
        return _fallback(x, level)
    low, high, _ = _run_device(x)
    return low, high


# revision 23
# speedup vs baseline: 1.3285x; 1.2266x over previous
"""HaarWavelet2D (level=2) Trainium2 kernel, v3.

Contract: kernel(x, level) with x [8, 64, 256, 256] fp32, level=2.
Returns (low_freq, high_freq), each [8, 64, 256, 256] fp32.

Sharding: data-parallel over batch - core b processes x[b] (64 channels).

v3 design (validated in model3.py):
- Host pre-converts x to bf16 and pre-splits even/odd columns:
  DRAM layout [C, 128(k), 2(rp), 2(eo), 128(j)], row = 2k+rp, col = 2j+eo.
  All on-chip ops become unit-stride (2x DVE mode); no casting DMAs.
- Vertical linear ops on the PE: t1 = D@s (drained with fused Abs -> a1),
  qL = Va@s (row-pair sum folded), qh = Vh@ch0, qlo = (0.1875*V128@S2)@s2,
  qh1 = (0.1875*V128)@ch1.  All scale factors folded into weights/drains.
- abs_max ALU op computes m = max(|d[r]|,|d[r+1]|) without materializing |d|.
- Row shift for m's odd rows via one small SBUF->SBUF DMA of d (rp0).
- Horizontal resizes elementwise with per-column weight tile w0t.
- Final 2x column upsample via stt with immediate 1/3 scalar + pad columns.
- Outputs stored in split-column layout; host re-interleaves.
"""

import sys

if "/opt/trn_rl_repo" not in sys.path:
    sys.path.insert(0, "/opt/trn_rl_repo")

import numpy as np
import ml_dtypes

BF = ml_dtypes.bfloat16

B_, C_, H_, W_ = 8, 64, 256, 256
NCORES = 8
G = 4            # channels per group
NCHUNK = 2      # channels per level-0 matmul chunk (N = 2*2*128 = 512)
P = 128


# ----------------------------------------------------------------------------
# host-side weight construction
# ----------------------------------------------------------------------------

def _resize_matrix(n, N):
    M = np.zeros((N, n), dtype=np.float64)
    for i in range(N):
        c = (i + 0.5) * n / N - 0.5
        j0 = int(np.floor(c))
        f = c - j0
        M[i, min(max(j0, 0), n - 1)] += 1.0 - f
        M[i, min(max(j0 + 1, 0), n - 1)] += f
    return M


def _build_weights():
    V255 = _resize_matrix(255, 256)
    V128 = _resize_matrix(128, 256)
    Sv1 = np.zeros((255, 256))
    for r in range(255):
        Sv1[r, r] = Sv1[r, r + 1] = 1.0
    S2 = np.zeros((128, 256))
    for k in range(128):
        S2[k, 2 * k] = S2[k, 2 * k + 1] = 1.0
    D = np.zeros((255, 256))
    for r in range(255):
        D[r, r] = 1.0
        D[r, r + 1] = -1.0

    Va = 0.25 * (V255 @ Sv1)                       # [256,256]
    Vh = np.zeros((256, 256))
    Vh[:, 0:255] = 0.25 * V255
    Dm = np.zeros((256, 256))
    Dm[0:255, :] = D
    Wlo = 0.1875 * (V128 @ S2)                     # [256,256]
    Wh1 = 0.1875 * V128                            # [256,128]

    w0 = np.zeros(256)
    for i in range(1, 256):
        w0[i] = V255[i, i - 1]
    w0[255] = 1.0

    w = {}
    for p in range(2):
        for rp in range(2):
            w[f"va_{p}{rp}"] = Va[p::2, rp::2].T
            w[f"vh_{p}{rp}"] = Vh[p::2, rp::2].T
            w[f"dm_{p}{rp}"] = Dm[p::2, rp::2].T
            w[f"wlo_{p}{rp}"] = Wlo[p::2, rp::2].T
        w[f"wh1_{p}"] = Wh1[p::2, :].T
    w0eo = np.stack([w0[0::2], w0[1::2]])          # [2,128]
    w["w0t"] = np.ascontiguousarray(
        np.broadcast_to(w0eo[None, None, None], (P, G, 2, 2, 128)))
    return {k: v.astype(BF) for k, v in w.items()}


_WEIGHTS = None


def _weights():
    global _WEIGHTS
    if _WEIGHTS is None:
        _WEIGHTS = _build_weights()
    return _WEIGHTS


# ----------------------------------------------------------------------------
# bass program
# ----------------------------------------------------------------------------

_NC_CACHE = {}


def build_nc(C=C_):
    key = C
    if key in _NC_CACHE:
        return _NC_CACHE[key]

    import concourse.bass as bass
    import concourse.bacc as bacc
    import concourse.tile as tile
    import concourse.mybir as mybir

    F32 = mybir.dt.float32
    BF16 = mybir.dt.bfloat16
    Alu = mybir.AluOpType
    Act = mybir.ActivationFunctionType

    nc = bacc.Bacc("TRN2", target_bir_lowering=False)
    x_d = nc.dram_tensor("x", [C, P, 2, 2, 128], BF16, kind="ExternalInput")
    wt = _weights()
    w_d = {
        name: nc.dram_tensor(name, list(arr.shape), BF16, kind="ExternalInput")
        for name, arr in wt.items()
    }
    low_d = nc.dram_tensor("low", [C, P, 2, 2, 128], BF16, kind="ExternalOutput")
    high_d = nc.dram_tensor("high", [C, P, 2, 2, 128], BF16,
                            kind="ExternalOutput")

    n_iter = C // G

    with tile.TileContext(nc) as tc:
        with (
            tc.tile_pool(name="consts", bufs=1) as consts,
            tc.tile_pool(name="qp", bufs=1) as qp,
            tc.tile_pool(name="xin", bufs=3) as xin,
            tc.tile_pool(name="mid", bufs=2) as mid,
            tc.tile_pool(name="lv1", bufs=2) as lv1,
            tc.tile_pool(name="outp", bufs=2) as outp,
            tc.tile_pool(name="ps", bufs=1, space="PSUM") as ps,
        ):
            wtile = {}
            for name, arr in wt.items():
                t = consts.tile(list(arr.shape), BF16, tag=name)
                nc.sync.dma_start(out=t, in_=w_d[name][...])
                wtile[name] = t
            w0t = wtile["w0t"]

            # persistent slotted tiles (stable addresses for one-time pad
            # memsets; slot = it % 2)
            s2t = qp.tile([P, 2, G, 2, 2, 128], BF16, tag="s")
            d2t = qp.tile([P, 2, G, 2, 2, 128], BF16, tag="d")
            dsh2t = qp.tile([P, 2, G, 2, 128], BF16, tag="dsh")
            dTL2t = qp.tile([P, 2, G, 2, 2, 128], BF16, tag="dTL")
            dTh2t = qp.tile([P, 2, G, 2, 2, 128], BF16, tag="dTh")

            # dsh partition 127 must stay 0; DMA writes only 0:127 each group
            nc.vector.memset(dsh2t, 0.0)
            for sl in range(2):
                nc.vector.memset(s2t[:, sl, :, :, 1, 127:128], 0.0)
                nc.vector.memset(d2t[:, sl, :, :, 1, 127:128], 0.0)
                nc.vector.memset(dTL2t[:, sl, :, :, 0, 0:1], 0.0)
                nc.vector.memset(dTh2t[:, sl, :, :, 0, 0:1], 0.0)

            # PSUM: 8 banks as 8 [128,512] f32 tiles
            psA = [ps.tile([P, 512], F32, tag=f"psA{i}", name=f"psA{i}")
                   for i in range(4)]
            psB = [ps.tile([P, 512], F32, tag=f"psB{i}", name=f"psB{i}")
                   for i in range(4)]

            def stage_load(it):
                c0 = it * G
                X = xin.tile([P, G, 2, 2, 128], BF16, tag="X")
                nc.sync.dma_start(
                    out=X,
                    in_=x_d[c0:c0 + G].rearrange("c k p e j -> k c p e j"))
                return X

            def mm_quant(wpfx, rhs_of, nchunks, psl, accum_rp=True):
                """Emit LDW-efficient matmuls: for p, rp: for chunk."""
                for p in range(2):
                    if accum_rp:
                        for rp in range(2):
                            for ci in range(nchunks):
                                nc.tensor.matmul(
                                    out=psl[p * nchunks + ci][:, 0:512],
                                    lhsT=wtile[f"{wpfx}_{p}{rp}"][...],
                                    rhs=rhs_of(ci, rp),
                                    start=(rp == 0), stop=(rp == 1))
                    else:
                        for ci in range(nchunks):
                            nc.tensor.matmul(
                                out=psl[p * nchunks + ci][:, 0:512],
                                lhsT=wtile[f"{wpfx}_{p}"][...],
                                rhs=rhs_of(ci, 0),
                                start=True, stop=True)

            def stage_front_a(it, X):
                sl = it % 2
                s = s2t[:, sl]
                d = d2t[:, sl]
                dsh = dsh2t[:, sl]

                # s/d column-pair sums/diffs, split-column layout
                nc.vector.tensor_tensor(
                    out=s[:, :, :, 0], in0=X[:, :, :, 0], in1=X[:, :, :, 1],
                    op=Alu.add)
                nc.vector.tensor_tensor(
                    out=s[:, :, :, 1, 0:127], in0=X[:, :, :, 1, 0:127],
                    in1=X[:, :, :, 0, 1:128], op=Alu.add)
                nc.vector.tensor_tensor(
                    out=d[:, :, :, 0], in0=X[:, :, :, 0], in1=X[:, :, :, 1],
                    op=Alu.subtract)
                nc.vector.tensor_tensor(
                    out=d[:, :, :, 1, 0:127], in0=X[:, :, :, 1, 0:127],
                    in1=X[:, :, :, 0, 1:128], op=Alu.subtract)

                # ad = |d| (scalar; first op of this slot's scalar segment),
                # row-shifted rp0 slice for the odd rows
                ad = mid.tile([P, G, 2, 2, 128], BF16, tag="ad")
                nc.scalar.activation(out=ad, in_=d, func=Act.Abs)
                nc.sync.dma_start(out=dsh[0:127], in_=ad[1:128, :, 0])

                # t1 = D @ s on PE, drained with Abs*0.5 -> a1
                a1 = mid.tile([P, G, 2, 2, 128], BF16, tag="a1")
                mm_quant("dm", lambda ci, rp: s2t[:, sl, 2 * ci:2 * ci + 2, rp],
                         NCHUNK, psA)
                for p in range(2):
                    for ci in range(NCHUNK):
                        nc.scalar.activation(
                            out=a1[:, 2 * ci:2 * ci + 2, p],
                            in_=psA[p * NCHUNK + ci][:, 0:512].rearrange(
                                "r (c e j) -> r c e j", c=2, e=2),
                            func=Act.Abs, scale=0.5)
                return ad, a1

            def stage_vert_a(it):
                sl = it % 2
                qL = mid.tile([P, G, 2, 2, 128], BF16, tag="qL")
                mm_quant("va", lambda ci, rp: s2t[:, sl, 2 * ci:2 * ci + 2, rp],
                         NCHUNK, psB)
                for p in range(2):
                    for ci in range(NCHUNK):
                        nc.scalar.copy(
                            out=qL[:, 2 * ci:2 * ci + 2, p],
                            in_=psB[p * NCHUNK + ci][:, 0:512].rearrange(
                                "r (c e j) -> r c e j", c=2, e=2))
                return qL

            def stage_front_b(it, ad):
                sl = it % 2
                dsh = dsh2t[:, sl]
                m = mid.tile([P, G, 2, 2, 128], BF16, tag="m")
                nc.vector.tensor_tensor(
                    out=m[:, :, 0], in0=ad[:, :, 0], in1=ad[:, :, 1],
                    op=Alu.max)
                nc.vector.tensor_tensor(
                    out=m[:, :, 1], in0=ad[:, :, 1], in1=dsh, op=Alu.max)
                return m

            def stage_vert_b(it, a1, m):
                # qh = Vh @ (a1 + m), summed via PSUM accumulation
                qh = mid.tile([P, G, 2, 2, 128], BF16, tag="qh")
                for p in range(2):
                    for rp in range(2):
                        for ci in range(NCHUNK):
                            for si, src in enumerate((a1, m)):
                                nc.tensor.matmul(
                                    out=psB[p * NCHUNK + ci][:, 0:512],
                                    lhsT=wtile[f"vh_{p}{rp}"][...],
                                    rhs=src[:, 2 * ci:2 * ci + 2, rp],
                                    start=(rp == 0 and si == 0),
                                    stop=(rp == 1 and si == 1))
                for p in range(2):
                    for ci in range(NCHUNK):
                        nc.scalar.copy(
                            out=qh[:, 2 * ci:2 * ci + 2, p],
                            in_=psB[p * NCHUNK + ci][:, 0:512].rearrange(
                                "r (c e j) -> r c e j", c=2, e=2))
                return qh

            def _hres(it, q, dT2t, tmptag, outtag):
                sl = it % 2
                dT = dT2t[:, sl]
                out = mid.tile([P, G, 2, 2, 128], BF16, tag=outtag,
                               name=outtag)
                nc.vector.tensor_tensor(
                    out=dT[:, :, :, 0, 1:128], in0=q[:, :, :, 1, 0:127],
                    in1=q[:, :, :, 0, 1:128], op=Alu.subtract)
                nc.vector.tensor_tensor(
                    out=dT[:, :, :, 1], in0=q[:, :, :, 0],
                    in1=q[:, :, :, 1], op=Alu.subtract)
                tmp = mid.tile([P, G, 2, 2, 128], BF16, tag=tmptag,
                               name=tmptag)
                nc.vector.tensor_tensor(out=tmp, in0=dT, in1=w0t, op=Alu.mult)
                nc.vector.tensor_tensor(out=out, in0=q, in1=tmp, op=Alu.add)
                return out

            def stage_horiz_L(it, qL):
                return _hres(it, qL, dTL2t, "tmpL", "L0x")

            def stage_horiz_h(it, qh):
                return _hres(it, qh, dTh2t, "tmpH", "h0x")

            def stage_lv1_front(it, L0x):
                s2 = lv1.tile([P, G, 2, 128], BF16, tag="s2")
                d2 = lv1.tile([P, G, 2, 128], BF16, tag="d2")
                nc.vector.tensor_tensor(
                    out=s2, in0=L0x[:, :, :, 0], in1=L0x[:, :, :, 1],
                    op=Alu.add)
                nc.vector.tensor_tensor(
                    out=d2, in0=L0x[:, :, :, 0], in1=L0x[:, :, :, 1],
                    op=Alu.subtract)
                t1b = lv1.tile([P, G, 128], BF16, tag="t1b")
                a1b = lv1.tile([P, G, 128], BF16, tag="a1b")
                ad2 = lv1.tile([P, G, 2, 128], BF16, tag="ad2")
                m1 = lv1.tile([P, G, 128], BF16, tag="m1")
                ch1 = lv1.tile([P, G, 128], BF16, tag="ch1")
                nc.vector.tensor_tensor(
                    out=t1b, in0=s2[:, :, 0], in1=s2[:, :, 1], op=Alu.subtract)
                nc.scalar.activation(out=a1b, in_=t1b, func=Act.Abs, scale=0.5)
                nc.scalar.activation(out=ad2, in_=d2, func=Act.Abs)
                nc.vector.tensor_tensor(
                    out=m1, in0=ad2[:, :, 0], in1=ad2[:, :, 1], op=Alu.max)
                nc.vector.tensor_tensor(out=ch1, in0=a1b, in1=m1, op=Alu.add)
                return s2, ch1

            def stage_lv1_mm(it, s2, ch1):
                qlo = lv1.tile([P, G, 2, 130], BF16, tag="qlo")
                qh1 = lv1.tile([P, G, 2, 130], BF16, tag="qh1")
                mm_quant("wlo", lambda ci, rp: s2[:, :, rp], 1, psA[0:2])
                mm_quant("wh1", lambda ci, rp: ch1, 1, psA[2:4],
                         accum_rp=False)
                for p in range(2):
                    for src, dst in ((psA[p], qlo), (psA[2 + p], qh1)):
                        sv = src[:, 0:512].rearrange("r (g j) -> r g j", j=128)
                        nc.scalar.copy(out=dst[:, :, p, 1:129], in_=sv)
                        # both edge pad cols in one strided drain
                        nc.scalar.copy(out=dst[:, :, p, 0:130:129],
                                       in_=sv[:, :, 0:128:127])
                return qlo, qh1

            def stage_final(it, qlo, qh1, h0x):
                c0 = it * G
                low = outp.tile([P, G, 2, 2, 128], BF16, tag="low")
                hh = outp.tile([P, G, 2, 2, 128], BF16, tag="hh")
                high = outp.tile([P, G, 2, 2, 128], BF16, tag="high")
                for q, out in ((qlo, low), (qh1, hh)):
                    nc.vector.scalar_tensor_tensor(
                        out=out[:, :, :, 0], in0=q[:, :, :, 0:128],
                        scalar=1.0 / 3.0, in1=q[:, :, :, 1:129],
                        op0=Alu.mult, op1=Alu.add)
                    nc.vector.scalar_tensor_tensor(
                        out=out[:, :, :, 1], in0=q[:, :, :, 2:130],
                        scalar=1.0 / 3.0, in1=q[:, :, :, 1:129],
                        op0=Alu.mult, op1=Alu.add)
                nc.gpsimd.tensor_tensor(out=high, in0=h0x, in1=hh, op=Alu.add)
                nc.scalar.dma_start(
                    out=low_d[c0:c0 + G].rearrange("c k p e j -> k c p e j"),
                    in_=low)
                nc.scalar.dma_start(
                    out=high_d[c0:c0 + G].rearrange("c k p e j -> k c p e j"),
                    in_=high)

            # Skewed-stage pipeline: in emission slot `it`, the vector queue
            # runs [s/d(it), horiz(it-1), m(it), lv1_front(it-1),
            # final-stts(it-2)] so the ad(it)->dsh(it) chain (gpsimd+DMA)
            # and all PE/scalar drains have a queue of independent work to
            # hide behind.
            xt = {k: stage_load(k) for k in range(2)}
            st = {}
            for it in range(n_iter + 3):
                if it < n_iter:
                    if it + 2 < n_iter:
                        xt[it + 2] = stage_load(it + 2)
                    ad, a1 = stage_front_a(it, xt.pop(it))
                    st[it] = {"ad": ad, "a1": a1, "qL": stage_vert_a(it)}
                if 1 <= it <= n_iter:
                    g = st[it - 1]
                    m = stage_front_b(it - 1, g["ad"])
                    g["qh"] = stage_vert_b(it - 1, g["a1"], m)
                    g["L0x"] = stage_horiz_L(it - 1, g["qL"])
                if 2 <= it <= n_iter + 1:
                    g = st[it - 2]
                    g["h0x"] = stage_horiz_h(it - 2, g["qh"])
                    g["s2"], g["ch1"] = stage_lv1_front(it - 2, g["L0x"])
                if it >= 3:
                    g = st.pop(it - 3)
                    qlo, qh1 = stage_lv1_mm(it - 3, g["s2"], g["ch1"])
                    stage_final(it - 3, qlo, qh1, g["h0x"])

    nc.compile()
    _NC_CACHE[key] = nc
    return nc


# ----------------------------------------------------------------------------
# host entry points
# ----------------------------------------------------------------------------

def _host_prep(x):
    """x [B, C, H, W] f32 -> [B*C, 128, 2, 2, 128] bf16 split-column."""
    xb = x.reshape(B_ * C_, P, 2, 128, 2).astype(BF)
    return np.ascontiguousarray(xb.transpose(0, 1, 2, 4, 3))


def _host_post(out):
    """[B*C, 128, 2, 2, 128] bf16 -> [B, C, H, W] f32 interleaved."""
    o = np.asarray(out).transpose(0, 1, 2, 4, 3)
    return np.ascontiguousarray(o).reshape(B_, C_, H_, W_).astype(np.float32)


_RUNNER = None


def _get_runner():
    """Builds (once) a cached sharded jit executable over the 8 cores."""
    global _RUNNER
    if _RUNNER is not None:
        return _RUNNER

    import jax
    from jax.sharding import Mesh, PartitionSpec, NamedSharding
    from jax.experimental.shard_map import shard_map
    import concourse.mybir as mybir
    from concourse import bass2jax
    from concourse.bass2jax import _bass_exec_p, partition_id_tensor

    bass2jax.install_neuronx_cc_hook()
    nc = build_nc(C_)

    partition_name = nc.partition_id_tensor.name if nc.partition_id_tensor else None
    in_names, out_names, out_avals = [], [], []
    for alloc in nc.m.functions[0].allocations:
        if not isinstance(alloc, mybir.MemoryLocationSet):
            continue
        name = alloc.memorylocations[0].name
        if alloc.kind == "ExternalInput":
            if name != partition_name:
                in_names.append(name)
        elif alloc.kind == "ExternalOutput":
            out_names.append(name)
            out_avals.append(jax.core.ShapedArray(
                tuple(alloc.tensor_shape), mybir.dt.np(alloc.dtype)))
    n_params = len(in_names)
    all_in_names = list(in_names) + list(out_names)
    if partition_name is not None:
        all_in_names.append(partition_name)

    def _body(*args):
        operands = list(args)
        if partition_name is not None:
            operands.append(partition_id_tensor())
        return tuple(_bass_exec_p.bind(
            *operands,
            out_avals=tuple(out_avals),
            in_names=tuple(all_in_names),
            out_names=tuple(out_names),
            lowering_input_output_aliases=(),
            sim_require_finite=True,
            sim_require_nnan=True,
            nc=nc,
        ))

    devices = jax.devices()[:NCORES]
    mesh = Mesh(np.asarray(devices), ("core",))
    n_in = n_params + len(out_names)
    sharded = jax.jit(shard_map(
        _body, mesh=mesh,
        in_specs=(PartitionSpec("core"),) * n_in,
        out_specs=(PartitionSpec("core"),) * len(out_names),
        check_rep=False))

    shard0 = NamedSharding(mesh, PartitionSpec("core"))
    wt = _weights()
    static = {}
    for name in in_names:
        if name == "x":
            continue
        arr = np.concatenate([wt[name]] * NCORES, axis=0)
        static[name] = jax.device_put(arr, shard0)
    for name, aval in zip(out_names, out_avals):
        z = np.zeros((aval.shape[0] * NCORES,) + tuple(aval.shape[1:]),
                     dtype=aval.dtype)
        static[name] = jax.device_put(z, shard0)

    def run(x_global):
        ops = []
        for name in in_names:
            ops.append(x_global if name == "x" else static[name])
        for name in out_names:
            ops.append(static[name])
        outs = sharded(*ops)
        return dict(zip(out_names, outs))

    _RUNNER = (run, shard0)
    return _RUNNER


def _run_device(x, trace=False):
    """x: [8, 64, 256, 256] fp32. Returns (low, high, results_obj)."""
    xb = _host_prep(x)
    if trace:
        from concourse import bass_utils
        nc = build_nc(C_)
        wt = _weights()
        in_maps = [
            dict(wt, x=xb[b * C_:(b + 1) * C_])
            for b in range(NCORES)
        ]
        res = bass_utils.run_bass_kernel_spmd(
            nc, in_maps, core_ids=list(range(NCORES)), trace=True)
        low = np.stack([res.results[b]["low"] for b in range(NCORES)])
        high = np.stack([res.results[b]["high"] for b in range(NCORES)])
        low = low.reshape(B_ * C_, P, 2, 2, 128)
        high = high.reshape(B_ * C_, P, 2, 2, 128)
        return _host_post(low), _host_post(high), res

    run, _ = _get_runner()
    outs = run(xb)
    return (_host_post(outs["low"]), _host_post(outs["high"]), None)


def _fallback(x, level):
    """Numpy port of the reference for unexpected shapes/levels."""
    xl = x.astype(np.float64)
    low = xl
    high = np.zeros_like(xl)
    Bb, Cc, H, W = xl.shape

    def up(a, n_r, n_c):
        Mr = _resize_matrix(a.shape[-2], n_r)
        Mc = _resize_matrix(a.shape[-1], n_c)
        return np.einsum("ij,...jk,lk->...il", Mr, a, Mc)

    for lv in range(level):
        stride = 2 ** lv
        if H // stride < 2 or W // stride < 2:
            break
        x00 = low[..., 0:H - 1:stride, 0:W - 1:stride]
        x01 = low[..., 0:H - 1:stride, 1:W:stride]
        x10 = low[..., 1:H:stride, 0:W - 1:stride]
        x11 = low[..., 1:H:stride, 1:W:stride]
        ll = (x00 + x01 + x10 + x11) * 0.25
        lh = (x00 + x01 - x10 - x11) * 0.25
        hl = (x00 - x01 + x10 - x11) * 0.25
        hh = (x00 - x01 - x10 + x11) * 0.25
        ch = np.abs(lh) + np.abs(hl) + np.abs(hh)
        high = high + up(ch, H, W)
        low = up(ll, H, W)
    if level > 0:
        high = high / level
    return low.astype(np.float32), high.astype(np.float32)


def kernel(x, level):
    x = np.asarray(x, dtype=np.float32)
    level = int(level)
    if level != 2 or x.shape != (B_, C_, H_, W_):
        return _fallback(x, level)
    low, high, _ = _run_device(x)
    return low, high
